# revision 6
# baseline (speedup 1.0000x reference)
"""Butterfly transform kernel for Trainium2 (8 NeuronCores, SPMD data parallel).

Math: reference applies 12 butterfly layers; every layer pairs the SAME
adjacent columns (2n, 2n+1) and multiplies each pair by a per-pair 2x2
matrix W[l, n].  The composition therefore collapses into a single per-pair
2x2 matrix  M[n] = W[0,n] @ W[1,n] @ ... @ W[11,n], so the device kernel is
ONE memory-bound pass over x instead of 12:

    y[:, 2n]   = x[:, 2n] * M[n,0,0] + x[:, 2n+1] * M[n,1,0]
    y[:, 2n+1] = x[:, 2n] * M[n,0,1] + x[:, 2n+1] * M[n,1,1]

Device layout: batch rows in SBUF partitions, features along free dim.
fp32 tensor_tensor on the DVE runs at 1x (4096+151 cyc @0.96 GHz), making
the 3-op/tile fp32 pipeline DVE-bound at ~106us — above the ~94us HBM
roofline (32 MiB compulsory read+write per core @ ~358 GB/s).  The 16-bit
path gets the DVE out of the way:

  - loads cast f32->f16 in the DMA itself (SWDGE casts in-flight),
  - the pair-swap z = pairswap(x) runs as a strided copy on the idle
    ScalarE (1x, any stride),
  - the three DVE ops (y=x*A, z*=B', y+=z) are all contiguous 16-bit
    tensor_tensor -> 2x_1P mode, ~55us/pass total,
  - stores cast f16->f32 in the DMA.

Weights are packed host-side as full-width A (diag, interleaved per pair)
and B' (cross terms pre-swapped: B'[2n]=M[n,1,0], B'[2n+1]=M[n,0,1]) so
y = x*A + pairswap(x)*B' with every DVE operand contiguous.
"""

import sys
import numpy as np

if "/opt/trn_rl_repo" not in sys.path:
    sys.path.insert(0, "/opt/trn_rl_repo")

BATCH = 8192
SIZE = 4096
LOG_N = 12
HALF = SIZE // 2  # 2048
N_CORES = 8
ROWS_PER_CORE = BATCH // N_CORES  # 1024
P = 128  # SBUF partitions
N_TILES = ROWS_PER_CORE // P  # 8

DEFAULT_MODE = "dma"
DEFAULT_CDT = "float16"

_CACHE = {}


def _build_program(
    repeats: int = 1,
    mode: str = DEFAULT_MODE,
    cdt_name: str = DEFAULT_CDT,
    xio_bufs: int = 4,
    zio_bufs: int = 3,
    yio_bufs: int = 3,
):
    import concourse.bass as bass
    import concourse.bacc as bacc
    import concourse.mybir as mybir
    from concourse import tile
    from contextlib import ExitStack

    f32 = mybir.dt.float32
    cdt = getattr(mybir.dt, cdt_name)
    nc = bacc.Bacc(None, num_swdge_queues=4)

    x_in = nc.dram_tensor("x", [ROWS_PER_CORE, SIZE], f32, kind="ExternalInput")
    w_in = nc.dram_tensor("wf", [1, 2 * SIZE], f32, kind="ExternalInput")
    y_out = nc.dram_tensor("y", [ROWS_PER_CORE, SIZE], f32, kind="ExternalOutput")

    mult = mybir.AluOpType.mult
    add = mybir.AluOpType.add

    with tile.TileContext(nc) as tc, ExitStack() as ctx:
        const = ctx.enter_context(tc.tile_pool(name="const", bufs=1))
        xio = ctx.enter_context(tc.tile_pool(name="xio", bufs=xio_bufs))
        zio = ctx.enter_context(tc.tile_pool(name="zio", bufs=zio_bufs))
        yio = ctx.enter_context(tc.tile_pool(name="yio", bufs=yio_bufs))
        psum = ctx.enter_context(tc.tile_pool(name="psum", bufs=2, space="PSUM"))

        # --- prologue: broadcast the packed weight row to all 128 partitions
        # in the compute dtype.  One casting DMA brings the row into
        # partition 0; the ones-matmul on the (idle) PE replicates it across
        # partitions; ACT copies PSUM->SBUF with the f32->cdt cast.
        needs_w = mode in ("dma", "eng", "f32")
        if needs_w:
            wdt = f32 if mode == "f32" else cdt
            ones = const.tile([1, P], wdt)
            nc.vector.memset(ones[:], 1.0)
            wrow = const.tile([1, 2 * SIZE], wdt)
            if wdt == f32:
                nc.sync.dma_start(wrow[:], w_in[:])
            else:
                nc.gpsimd.dma_start(wrow[:], w_in[:])  # casting load
            wc = const.tile([P, 2 * SIZE], wdt)
            for c in range(4):
                pt = psum.tile([P, HALF], f32, tag="wpsum")
                for j in range(HALF // 512):
                    nc.tensor.matmul(
                        pt[:, j * 512 : (j + 1) * 512],
                        ones[:],
                        wrow[:, c * HALF + j * 512 : c * HALF + (j + 1) * 512],
                        start=True,
                        stop=True,
                    )
                nc.scalar.copy(wc[:, c * HALF : (c + 1) * HALF], pt[:])
            a_w = wc[:, 0:SIZE]  # [M00 M11 M00 M11 ...] interleaved per pair
            b_w = wc[:, SIZE:]   # [M10 M01 M10 M01 ...] pre-swapped cross terms

        for i in range(N_TILES * repeats):
            i = i % N_TILES
            rows = slice(i * P, (i + 1) * P)

            if mode == "copy":
                # DMA-roofline probe: f32 in/out, no compute.
                xt = xio.tile([P, SIZE], f32, tag="xt")
                nc.sync.dma_start(xt[:], x_in[rows, :])
                nc.gpsimd.dma_start(y_out[rows, :], xt[:])
                continue

            if mode == "copy16":
                # cast-DMA roofline probe: f32->cdt load, cdt->f32 store.
                xb = xio.tile([P, SIZE], cdt, tag="xb")
                nc.gpsimd.dma_start(xb[:], x_in[rows, :])
                nc.gpsimd.dma_start(y_out[rows, :], xb[:])
                continue

            if mode == "dma":
                # 16-bit compute, casts ride the DMAs.
                xb = xio.tile([P, SIZE], cdt, tag="xb")
                nc.gpsimd.dma_start(xb[:], x_in[rows, :])
                zb = zio.tile([P, SIZE], cdt, tag="zb")
                z3 = zb[:].rearrange("p (n two) -> p n two", two=2)
                x3 = xb[:].rearrange("p (n two) -> p n two", two=2)[:, :, ::-1]
                nc.scalar.copy(z3, x3)  # pairswap on the idle ScalarE
                yb = yio.tile([P, SIZE], cdt, tag="yb")
                nc.vector.tensor_tensor(yb[:], xb[:], a_w, mult)
                nc.vector.tensor_tensor(zb[:], zb[:], b_w, mult)
                nc.vector.tensor_tensor(yb[:], yb[:], zb[:], add)
                nc.gpsimd.dma_start(y_out[rows, :], yb[:])
                continue

            if mode == "eng":
                # fallback: plain f32 DMAs, casts on DVE/ScalarE.
                xt = xio.tile([P, SIZE], f32, tag="xt", bufs=3)
                nc.sync.dma_start(xt[:], x_in[rows, :])
                xb = xio.tile([P, SIZE], cdt, tag="xb", bufs=2)
                nc.vector.tensor_copy(xb[:], xt[:])  # f32 copy 2x_2P
                zb = zio.tile([P, SIZE], cdt, tag="zb", bufs=2)
                z3 = zb[:].rearrange("p (n two) -> p n two", two=2)
                x3 = xt[:].rearrange("p (n two) -> p n two", two=2)[:, :, ::-1]
                nc.scalar.copy(z3, x3)  # swap + cast on ScalarE
                yb = zio.tile([P, SIZE], cdt, tag="yb", bufs=2)
                nc.vector.tensor_tensor(yb[:], xb[:], a_w, mult)
                nc.vector.tensor_tensor(zb[:], zb[:], b_w, mult)
                nc.vector.tensor_tensor(yb[:], yb[:], zb[:], add)
                yt = yio.tile([P, SIZE], f32, tag="yt", bufs=3)
                nc.scalar.copy(yt[:], yb[:])  # cast back on ScalarE
                nc.gpsimd.dma_start(y_out[rows, :], yt[:])
                continue

            if mode == "f32":
                # legacy all-f32 path (baseline "swst" math, B' packing):
                # y = x*A; z = pairswap-copy(x) on ScalarE; z *= B'; y += z.
                xt = xio.tile([P, SIZE], f32, tag="xt", bufs=3)
                nc.sync.dma_start(xt[:], x_in[rows, :])
                zt = zio.tile([P, SIZE], f32, tag="zt", bufs=2)
                z3 = zt[:].rearrange("p (n two) -> p n two", two=2)
                x3 = xt[:].rearrange("p (n two) -> p n two", two=2)[:, :, ::-1]
                nc.scalar.copy(z3, x3)
                yt = yio.tile([P, SIZE], f32, tag="yt")
                nc.vector.tensor_tensor(yt[:], xt[:], a_w, mult)
                nc.vector.tensor_tensor(zt[:], zt[:], b_w, mult)
                nc.vector.tensor_tensor(yt[:], yt[:], zt[:], add)
                nc.gpsimd.dma_start(y_out[rows, :], yt[:])
                continue

            raise ValueError(mode)

    nc.compile()
    return nc


def _get_nc(repeats: int = 1, **kwargs):
    key = ("nc", repeats, tuple(sorted(kwargs.items())))
    if key not in _CACHE:
        _CACHE[key] = _build_program(repeats, **kwargs)
    return _CACHE[key]


def fold_weights(W: np.ndarray) -> np.ndarray:
    """Compose the 12 stacked per-pair 2x2 layers into one, in float64.

    Returns wf [1, 2*SIZE] float32: full-width A (diag: M00,M11 interleaved)
    followed by full-width B' (cross terms pre-swapped so that
    y = x*A + pairswap(x)*B'): B'[2n] = M[n,1,0], B'[2n+1] = M[n,0,1]."""
    Wd = W.astype(np.float64)  # [12, HALF, 2, 2]
    M = Wd[0]
    for l in range(1, Wd.shape[0]):
        M = np.einsum("nij,njk->nik", M, Wd[l])
    M = M.astype(np.float32)  # [HALF, 2, 2]
    a_full = np.stack([M[:, 0, 0], M[:, 1, 1]], axis=1).reshape(SIZE)
    b_full = np.stack([M[:, 1, 0], M[:, 0, 1]], axis=1).reshape(SIZE)
    wf = np.concatenate([a_full, b_full])
    return np.ascontiguousarray(wf.reshape(1, 2 * SIZE))


def _run(x: np.ndarray, W: np.ndarray, run_kwargs=None, **build_kwargs):
    """Shard, run on the 8 cores, gather. Returns (output, BassKernelResults)."""
    from concourse.bass_utils import run_bass_kernel_spmd

    assert x.shape == (BATCH, SIZE) and W.shape == (LOG_N, HALF, 2, 2)
    x = np.ascontiguousarray(x, dtype=np.float32)
    wf = fold_weights(np.asarray(W))

    nc = _get_nc(**build_kwargs)
    in_maps = [
        {"x": x[c * ROWS_PER_CORE : (c + 1) * ROWS_PER_CORE], "wf": wf}
        for c in range(N_CORES)
    ]
    res = run_bass_kernel_spmd(
        nc, in_maps, core_ids=list(range(N_CORES)), **(run_kwargs or {})
    )
    out = np.concatenate([res.results[c]["y"] for c in range(N_CORES)], axis=0)
    return out, res


def kernel(x: np.ndarray, W: np.ndarray) -> np.ndarray:
    return _run(x, W)[0]


# revision 17
# speedup vs baseline: 1.2803x; 1.2803x over previous
"""Butterfly transform kernel for Trainium2 (8 NeuronCores, SPMD data parallel).

Math: reference applies 12 butterfly layers; every layer pairs the SAME
adjacent columns (2n, 2n+1) and multiplies each pair by a per-pair 2x2
matrix W[l, n].  The composition therefore collapses into a single per-pair
2x2 matrix  M[n] = W[0,n] @ W[1,n] @ ... @ W[11,n], so the device kernel is
ONE memory-bound pass over x instead of 12:

    y[:, 2n]   = x[:, 2n] * M[n,0,0] + x[:, 2n+1] * M[n,1,0]
    y[:, 2n+1] = x[:, 2n] * M[n,0,1] + x[:, 2n+1] * M[n,1,1]

Device layout: batch rows in SBUF partitions, features along free dim.
The pass is HBM-bound: 32 MiB compulsory read+write per core per pass.
Measured on trn2 (interleaved differential, R=65 vs 513): a pure
DMA-copy probe, the all-f32 3-op pipeline, and this 16-bit pipeline all
land at ~104.5-105us/pass (~321 GB/s/core effective of the ~358 GB/s
HBM-per-NC limit) — the DMA wall.  The 16-bit path is chosen because its
engine ceilings sit far below that wall, so it keeps tracking the DMA
floor if conditions improve, where fp32 would bind on the DVE:

  - loads cast f32->f16 in the DMA itself (SWDGE casts in-flight, free:
    cast-copy probe == plain-copy probe),
  - loads are emitted PRE tiles ahead of stores (both live on SWDGE
    queue 0, which the Pool NX drains in program order — a store's sem
    wait would otherwise block later loads' descriptor generation),
  - the pair-swap z = pairswap(x) runs as a strided copy on the idle
    ScalarE (1x, any stride),
  - the three DVE ops (y=x*A, z*=B', y+=z) are all contiguous 16-bit
    tensor_tensor -> 2x_1P mode, ~55us/pass total (fp32 would be ~106us:
    fp32 tensor_tensor has no 2x uop),
  - stores cast f16->f32 in the DMA.

Weights are packed host-side as full-width A (diag, interleaved per pair)
and B' (cross terms pre-swapped: B'[2n]=M[n,1,0], B'[2n+1]=M[n,0,1]) so
y = x*A + pairswap(x)*B' with every DVE operand contiguous.
"""

import sys
import numpy as np

if "/opt/trn_rl_repo" not in sys.path:
    sys.path.insert(0, "/opt/trn_rl_repo")

BATCH = 8192
SIZE = 4096
LOG_N = 12
HALF = SIZE // 2  # 2048
N_CORES = 8
ROWS_PER_CORE = BATCH // N_CORES  # 1024
P = 128  # SBUF partitions
N_TILES = ROWS_PER_CORE // P  # 8

DEFAULT_MODE = "dma"
DEFAULT_CDT = "float16"

_CACHE = {}


def _build_program(
    repeats: int = 1,
    mode: str = DEFAULT_MODE,
    cdt_name: str = DEFAULT_CDT,
    xio_bufs: int = 4,
    zio_bufs: int = 3,
    yio_bufs: int = 3,
):
    import concourse.bass as bass
    import concourse.bacc as bacc
    import concourse.mybir as mybir
    from concourse import tile
    from contextlib import ExitStack

    f32 = mybir.dt.float32
    cdt = getattr(mybir.dt, cdt_name)
    nc = bacc.Bacc(None, num_swdge_queues=4)

    x_in = nc.dram_tensor("x", [ROWS_PER_CORE, SIZE], f32, kind="ExternalInput")
    w_in = nc.dram_tensor("wf", [1, 2 * SIZE], f32, kind="ExternalInput")
    y_out = nc.dram_tensor("y", [ROWS_PER_CORE, SIZE], f32, kind="ExternalOutput")

    mult = mybir.AluOpType.mult
    add = mybir.AluOpType.add

    with tile.TileContext(nc) as tc, ExitStack() as ctx:
        const = ctx.enter_context(tc.tile_pool(name="const", bufs=1))
        xio = ctx.enter_context(tc.tile_pool(name="xio", bufs=xio_bufs))
        zio = ctx.enter_context(tc.tile_pool(name="zio", bufs=zio_bufs))
        yio = ctx.enter_context(tc.tile_pool(name="yio", bufs=yio_bufs))
        psum = ctx.enter_context(tc.tile_pool(name="psum", bufs=2, space="PSUM"))

        # --- prologue: broadcast the packed weight row to all 128 partitions
        # in the compute dtype.  One casting DMA brings the row into
        # partition 0; the ones-matmul on the (idle) PE replicates it across
        # partitions; ACT copies PSUM->SBUF with the f32->cdt cast.
        needs_w = mode in ("dma", "dma2", "dmal", "eng", "f32")
        if needs_w:
            wdt = f32 if mode == "f32" else cdt
            ones = const.tile([1, P], wdt)
            nc.vector.memset(ones[:], 1.0)
            wrow = const.tile([1, 2 * SIZE], wdt)
            if wdt == f32:
                nc.sync.dma_start(wrow[:], w_in[:])
            else:
                nc.gpsimd.dma_start(wrow[:], w_in[:])  # casting load
            if mode == "dma2":
                # double-wide: weights duplicated per 2-block tile,
                # wc2 = [a|a|b'|b'] so the [128, 2*SIZE] flat ops see the
                # right weight under both row-blocks.
                wc = const.tile([P, 4 * SIZE], wdt)
                for c in range(4):
                    pt = psum.tile([P, HALF], f32, tag="wpsum")
                    for j in range(HALF // 512):
                        nc.tensor.matmul(
                            pt[:, j * 512 : (j + 1) * 512],
                            ones[:],
                            wrow[:, c * HALF + j * 512 : c * HALF + (j + 1) * 512],
                            start=True,
                            stop=True,
                        )
                    half_sel = c // 2  # 0: a-chunks, 1: b'-chunks
                    base = half_sel * 2 * SIZE + (c % 2) * HALF
                    nc.scalar.copy(wc[:, base : base + HALF], pt[:])
                    nc.scalar.copy(wc[:, base + SIZE : base + SIZE + HALF], pt[:])
                a_w = wc[:, 0 : 2 * SIZE]
                b_w = wc[:, 2 * SIZE :]
            else:
                wc = const.tile([P, 2 * SIZE], wdt)
                for c in range(4):
                    pt = psum.tile([P, HALF], f32, tag="wpsum")
                    for j in range(HALF // 512):
                        nc.tensor.matmul(
                            pt[:, j * 512 : (j + 1) * 512],
                            ones[:],
                            wrow[:, c * HALF + j * 512 : c * HALF + (j + 1) * 512],
                            start=True,
                            stop=True,
                        )
                    nc.scalar.copy(wc[:, c * HALF : (c + 1) * HALF], pt[:])
                a_w = wc[:, 0:SIZE]  # [M00 M11 ...] interleaved per pair
                b_w = wc[:, SIZE:]   # [M10 M01 ...] pre-swapped cross terms

        if mode == "dma2":
            # Double-wide: 2 row-blocks (256 rows) per tile -> 4 tiles/pass,
            # [128, 8192] flat 16-bit ops, 4 MiB (f32-side) casting DMAs.
            # DRAM view "(b p) s -> p (b s)": partition p holds rows
            # 256*i2 + p and 256*i2 + 128 + p back to back.
            S2 = 2 * SIZE
            NT2 = N_TILES // 2
            total2 = NT2 * repeats
            PRE2 = 2
            xb2s = {}

            def _cast_load2(j):
                jj = j % NT2
                xb = xio.tile([P, S2], cdt, tag="xb", bufs=PRE2 + 2, name="xb")
                src = x_in[jj * 2 * P : (jj + 1) * 2 * P, :].rearrange(
                    "(b p) s -> p b s", b=2
                )
                nc.gpsimd.dma_start(xb[:].rearrange("p (b s) -> p b s", b=2), src)
                xb2s[j] = xb

            for j in range(min(PRE2, total2)):
                _cast_load2(j)
            for i in range(total2):
                jj = i % NT2
                if i + PRE2 < total2:
                    _cast_load2(i + PRE2)
                xb = xb2s.pop(i)
                zb = zio.tile([P, S2], cdt, tag="zb", bufs=2)
                z3 = zb[:].rearrange("p (n two) -> p n two", two=2)
                x3 = xb[:].rearrange("p (n two) -> p n two", two=2)[:, :, ::-1]
                nc.scalar.copy(z3, x3)  # pairswap on the idle ScalarE
                yb = yio.tile([P, S2], cdt, tag="yb", bufs=2)
                nc.vector.tensor_tensor(yb[:], xb[:], a_w, mult)
                nc.vector.tensor_tensor(zb[:], zb[:], b_w, mult)
                nc.vector.tensor_tensor(yb[:], yb[:], zb[:], add)
                dst = y_out[jj * 2 * P : (jj + 1) * 2 * P, :].rearrange(
                    "(b p) s -> p b s", b=2
                )
                nc.gpsimd.dma_start(dst, yb[:].rearrange("p (b s) -> p b s", b=2))
            total = 0  # dma2 handled; skip the generic per-tile loop

        else:
            total = N_TILES * repeats
        PRE = 3  # SWDGE load-prefetch depth: plain gpsimd.dma_start is
        # pinned to SWDGE queue 0 and the Pool NX executes in program
        # order, so loads must be emitted AHEAD of the store whose sem
        # wait would otherwise block their descriptor generation.
        _xbs = {}

        def _cast_load(j):
            jj = j % N_TILES
            xb = xio.tile([P, SIZE], cdt, tag="xb", bufs=PRE + 2, name="xb")
            nc.gpsimd.dma_start(xb[:], x_in[jj * P : (jj + 1) * P, :])
            _xbs[j] = xb

        if mode in ("dma", "dmal", "copy16"):
            for j in range(min(PRE, total)):
                _cast_load(j)

        for i in range(total):
            rows = slice((i % N_TILES) * P, (i % N_TILES + 1) * P)

            if mode == "copy":
                # DMA-roofline probe: f32 in/out, no compute.
                xt = xio.tile([P, SIZE], f32, tag="xt")
                nc.sync.dma_start(xt[:], x_in[rows, :])
                nc.gpsimd.dma_start(y_out[rows, :], xt[:])
                continue

            if mode == "copy16":
                # cast-DMA roofline probe: f32->cdt load, cdt->f32 store.
                if i + PRE < total:
                    _cast_load(i + PRE)
                xb = _xbs.pop(i)
                nc.gpsimd.dma_start(y_out[rows, :], xb[:])
                continue

            if mode in ("dma", "dmal"):
                # 16-bit compute, casts ride the DMAs.
                if i + PRE < total:
                    _cast_load(i + PRE)
                xb = _xbs.pop(i)
                zb = zio.tile([P, SIZE], cdt, tag="zb")
                z3 = zb[:].rearrange("p (n two) -> p n two", two=2)
                x3 = xb[:].rearrange("p (n two) -> p n two", two=2)[:, :, ::-1]
                nc.scalar.copy(z3, x3)  # pairswap on the idle ScalarE
                if mode == "dma":
                    yb = yio.tile([P, SIZE], cdt, tag="yb")
                    nc.vector.tensor_tensor(yb[:], xb[:], a_w, mult)
                    nc.vector.tensor_tensor(zb[:], zb[:], b_w, mult)
                    nc.vector.tensor_tensor(yb[:], yb[:], zb[:], add)
                    nc.gpsimd.dma_start(y_out[rows, :], yb[:])
                else:
                    # dmal: final add writes f32 (1x DVE), store on HWDGE so
                    # the Pool stream carries only never-stalling loads.
                    yb = zio.tile([P, SIZE], cdt, tag="yb")
                    nc.vector.tensor_tensor(yb[:], xb[:], a_w, mult)
                    nc.vector.tensor_tensor(zb[:], zb[:], b_w, mult)
                    yt = yio.tile([P, SIZE], f32, tag="yt")
                    nc.vector.tensor_tensor(yt[:], yb[:], zb[:], add)
                    nc.sync.dma_start(y_out[rows, :], yt[:])
                continue

            if mode == "eng":
                # fallback: plain f32 DMAs, casts on DVE/ScalarE.
                xt = xio.tile([P, SIZE], f32, tag="xt", bufs=3)
                nc.sync.dma_start(xt[:], x_in[rows, :])
                xb = xio.tile([P, SIZE], cdt, tag="xb", bufs=2)
                nc.vector.tensor_copy(xb[:], xt[:])  # f32 copy 2x_2P
                zb = zio.tile([P, SIZE], cdt, tag="zb", bufs=2)
                z3 = zb[:].rearrange("p (n two) -> p n two", two=2)
                x3 = xt[:].rearrange("p (n two) -> p n two", two=2)[:, :, ::-1]
                nc.scalar.copy(z3, x3)  # swap + cast on ScalarE
                yb = zio.tile([P, SIZE], cdt, tag="yb", bufs=2)
                nc.vector.tensor_tensor(yb[:], xb[:], a_w, mult)
                nc.vector.tensor_tensor(zb[:], zb[:], b_w, mult)
                nc.vector.tensor_tensor(yb[:], yb[:], zb[:], add)
                yt = yio.tile([P, SIZE], f32, tag="yt", bufs=3)
                nc.scalar.copy(yt[:], yb[:])  # cast back on ScalarE
                # HWDGE store: keeps GPSIMD off the SBUF port pair that the
                # DVE 2x_2P cast op lock-holds.
                nc.sync.dma_start(y_out[rows, :], yt[:])
                continue

            if mode == "f32":
                # legacy all-f32 path (baseline "swst" math, B' packing):
                # y = x*A; z = pairswap-copy(x) on ScalarE; z *= B'; y += z.
                xt = xio.tile([P, SIZE], f32, tag="xt", bufs=3)
                nc.sync.dma_start(xt[:], x_in[rows, :])
                zt = zio.tile([P, SIZE], f32, tag="zt", bufs=2)
                z3 = zt[:].rearrange("p (n two) -> p n two", two=2)
                x3 = xt[:].rearrange("p (n two) -> p n two", two=2)[:, :, ::-1]
                nc.scalar.copy(z3, x3)
                yt = yio.tile([P, SIZE], f32, tag="yt")
                nc.vector.tensor_tensor(yt[:], xt[:], a_w, mult)
                nc.vector.tensor_tensor(zt[:], zt[:], b_w, mult)
                nc.vector.tensor_tensor(yt[:], yt[:], zt[:], add)
                nc.gpsimd.dma_start(y_out[rows, :], yt[:])
                continue

            raise ValueError(mode)

    nc.compile()
    return nc


def _get_nc(repeats: int = 1, **kwargs):
    key = ("nc", repeats, tuple(sorted(kwargs.items())))
    if key not in _CACHE:
        _CACHE[key] = _build_program(repeats, **kwargs)
    return _CACHE[key]


def fold_weights(W: np.ndarray) -> np.ndarray:
    """Compose the 12 stacked per-pair 2x2 layers into one, in float64.

    Returns wf [1, 2*SIZE] float32: full-width A (diag: M00,M11 interleaved)
    followed by full-width B' (cross terms pre-swapped so that
    y = x*A + pairswap(x)*B'): B'[2n] = M[n,1,0], B'[2n+1] = M[n,0,1]."""
    Wd = W.astype(np.float64)  # [12, HALF, 2, 2]
    M = Wd[0]
    for l in range(1, Wd.shape[0]):
        M = np.einsum("nij,njk->nik", M, Wd[l])
    M = M.astype(np.float32)  # [HALF, 2, 2]
    a_full = np.stack([M[:, 0, 0], M[:, 1, 1]], axis=1).reshape(SIZE)
    b_full = np.stack([M[:, 1, 0], M[:, 0, 1]], axis=1).reshape(SIZE)
    wf = np.concatenate([a_full, b_full])
    return np.ascontiguousarray(wf.reshape(1, 2 * SIZE))


def _run(x: np.ndarray, W: np.ndarray, run_kwargs=None, **build_kwargs):
    """Shard, run on the 8 cores, gather. Returns (output, BassKernelResults)."""
    from concourse.bass_utils import run_bass_kernel_spmd

    assert x.shape == (BATCH, SIZE) and W.shape == (LOG_N, HALF, 2, 2)
    x = np.ascontiguousarray(x, dtype=np.float32)
    wf = fold_weights(np.asarray(W))

    nc = _get_nc(**build_kwargs)
    in_maps = [
        {"x": x[c * ROWS_PER_CORE : (c + 1) * ROWS_PER_CORE], "wf": wf}
        for c in range(N_CORES)
    ]
    res = run_bass_kernel_spmd(
        nc, in_maps, core_ids=list(range(N_CORES)), **(run_kwargs or {})
    )
    out = np.concatenate([res.results[c]["y"] for c in range(N_CORES)], axis=0)
    return out, res


def kernel(x: np.ndarray, W: np.ndarray) -> np.ndarray:
    return _run(x, W)[0]


# revision 20
# speedup vs baseline: 1.2886x; 1.0065x over previous
"""Butterfly transform kernel for Trainium2 (8 NeuronCores, SPMD data parallel).

Math: reference applies 12 butterfly layers; every layer pairs the SAME
adjacent columns (2n, 2n+1) and multiplies each pair by a per-pair 2x2
matrix W[l, n].  The composition therefore collapses into a single per-pair
2x2 matrix  M[n] = W[0,n] @ W[1,n] @ ... @ W[11,n], so the device kernel is
ONE memory-bound pass over x instead of 12:

    y[:, 2n]   = x[:, 2n] * M[n,0,0] + x[:, 2n+1] * M[n,1,0]
    y[:, 2n+1] = x[:, 2n] * M[n,0,1] + x[:, 2n+1] * M[n,1,1]

Device layout: batch rows in SBUF partitions, features along free dim.
The pass is HBM-bound: 32 MiB compulsory read+write per core per pass.
Measured on trn2 (interleaved differential, R=65 vs 513): a pure
DMA-copy probe, the all-f32 3-op pipeline, and this 16-bit pipeline all
land at ~104.5-105us/pass (~321 GB/s/core effective of the ~358 GB/s
HBM-per-NC limit) — the DMA wall.  The 16-bit path is chosen because its
engine ceilings sit far below that wall, so it keeps tracking the DMA
floor if conditions improve, where fp32 would bind on the DVE:

  - loads cast f32->f16 in the DMA itself (SWDGE casts in-flight, free:
    cast-copy probe == plain-copy probe),
  - loads are emitted PRE tiles ahead of stores (both live on SWDGE
    queue 0, which the Pool NX drains in program order — a store's sem
    wait would otherwise block later loads' descriptor generation),
  - the pair-swap z = pairswap(x) runs as a strided copy on the idle
    ScalarE (1x, any stride),
  - the three DVE ops (y=x*A, z*=B', y+=z) are all contiguous 16-bit
    tensor_tensor -> 2x_1P mode, ~55us/pass total (fp32 would be ~106us:
    fp32 tensor_tensor has no 2x uop),
  - stores cast f16->f32 in the DMA.

Weights are packed host-side as full-width A (diag, interleaved per pair)
and B' (cross terms pre-swapped: B'[2n]=M[n,1,0], B'[2n+1]=M[n,0,1]) so
y = x*A + pairswap(x)*B' with every DVE operand contiguous.
"""

import sys
import numpy as np

if "/opt/trn_rl_repo" not in sys.path:
    sys.path.insert(0, "/opt/trn_rl_repo")

BATCH = 8192
SIZE = 4096
LOG_N = 12
HALF = SIZE // 2  # 2048
N_CORES = 8
ROWS_PER_CORE = BATCH // N_CORES  # 1024
P = 128  # SBUF partitions
N_TILES = ROWS_PER_CORE // P  # 8

DEFAULT_MODE = "dma"
DEFAULT_CDT = "float16"

_CACHE = {}


def _build_program(
    repeats: int = 1,
    mode: str = DEFAULT_MODE,
    cdt_name: str = DEFAULT_CDT,
    xio_bufs: int = 4,
    zio_bufs: int = 3,
    yio_bufs: int = 3,
):
    import concourse.bass as bass
    import concourse.bacc as bacc
    import concourse.mybir as mybir
    from concourse import tile
    from contextlib import ExitStack

    f32 = mybir.dt.float32
    cdt = getattr(mybir.dt, cdt_name)
    nc = bacc.Bacc(None, num_swdge_queues=4)

    x_in = nc.dram_tensor("x", [ROWS_PER_CORE, SIZE], f32, kind="ExternalInput")
    w_in = nc.dram_tensor("wf", [1, 2 * SIZE], f32, kind="ExternalInput")
    y_out = nc.dram_tensor("y", [ROWS_PER_CORE, SIZE], f32, kind="ExternalOutput")

    mult = mybir.AluOpType.mult
    add = mybir.AluOpType.add

    with tile.TileContext(nc) as tc, ExitStack() as ctx:
        const = ctx.enter_context(tc.tile_pool(name="const", bufs=1))
        xio = ctx.enter_context(tc.tile_pool(name="xio", bufs=xio_bufs))
        zio = ctx.enter_context(tc.tile_pool(name="zio", bufs=zio_bufs))
        yio = ctx.enter_context(tc.tile_pool(name="yio", bufs=yio_bufs))
        psum = ctx.enter_context(tc.tile_pool(name="psum", bufs=2, space="PSUM"))

        # --- prologue: broadcast the packed weight row to all 128 partitions
        # in the compute dtype.  One casting DMA brings the row into
        # partition 0; the ones-matmul on the (idle) PE replicates it across
        # partitions; ACT copies PSUM->SBUF with the f32->cdt cast.
        needs_w = mode in ("dma", "dma2", "dmal", "eng", "f32", "hyb")
        if needs_w:
            wdt = f32 if mode == "f32" else cdt
            ones = const.tile([1, P], wdt)
            nc.vector.memset(ones[:], 1.0)
            wrow = const.tile([1, 2 * SIZE], wdt)
            if wdt == f32:
                nc.sync.dma_start(wrow[:], w_in[:])
            else:
                nc.gpsimd.dma_start(wrow[:], w_in[:])  # casting load
            if mode == "dma2":
                # double-wide: weights duplicated per 2-block tile,
                # wc2 = [a|a|b'|b'] so the [128, 2*SIZE] flat ops see the
                # right weight under both row-blocks.
                wc = const.tile([P, 4 * SIZE], wdt)
                for c in range(4):
                    pt = psum.tile([P, HALF], f32, tag="wpsum")
                    for j in range(HALF // 512):
                        nc.tensor.matmul(
                            pt[:, j * 512 : (j + 1) * 512],
                            ones[:],
                            wrow[:, c * HALF + j * 512 : c * HALF + (j + 1) * 512],
                            start=True,
                            stop=True,
                        )
                    half_sel = c // 2  # 0: a-chunks, 1: b'-chunks
                    base = half_sel * 2 * SIZE + (c % 2) * HALF
                    nc.scalar.copy(wc[:, base : base + HALF], pt[:])
                    nc.scalar.copy(wc[:, base + SIZE : base + SIZE + HALF], pt[:])
                a_w = wc[:, 0 : 2 * SIZE]
                b_w = wc[:, 2 * SIZE :]
            else:
                wc = const.tile([P, 2 * SIZE], wdt)
                for c in range(4):
                    pt = psum.tile([P, HALF], f32, tag="wpsum")
                    for j in range(HALF // 512):
                        nc.tensor.matmul(
                            pt[:, j * 512 : (j + 1) * 512],
                            ones[:],
                            wrow[:, c * HALF + j * 512 : c * HALF + (j + 1) * 512],
                            start=True,
                            stop=True,
                        )
                    nc.scalar.copy(wc[:, c * HALF : (c + 1) * HALF], pt[:])
                a_w = wc[:, 0:SIZE]  # [M00 M11 ...] interleaved per pair
                b_w = wc[:, SIZE:]   # [M10 M01 ...] pre-swapped cross terms

        if mode == "dma2":
            # Double-wide: 2 row-blocks (256 rows) per tile -> 4 tiles/pass,
            # [128, 8192] flat 16-bit ops, 4 MiB (f32-side) casting DMAs.
            # DRAM view "(b p) s -> p (b s)": partition p holds rows
            # 256*i2 + p and 256*i2 + 128 + p back to back.
            S2 = 2 * SIZE
            NT2 = N_TILES // 2
            total2 = NT2 * repeats
            PRE2 = 2
            xb2s = {}

            def _cast_load2(j):
                jj = j % NT2
                xb = xio.tile([P, S2], cdt, tag="xb", bufs=PRE2 + 2, name="xb")
                src = x_in[jj * 2 * P : (jj + 1) * 2 * P, :].rearrange(
                    "(b p) s -> p b s", b=2
                )
                nc.gpsimd.dma_start(xb[:].rearrange("p (b s) -> p b s", b=2), src)
                xb2s[j] = xb

            for j in range(min(PRE2, total2)):
                _cast_load2(j)
            for i in range(total2):
                jj = i % NT2
                if i + PRE2 < total2:
                    _cast_load2(i + PRE2)
                xb = xb2s.pop(i)
                zb = zio.tile([P, S2], cdt, tag="zb", bufs=2)
                z3 = zb[:].rearrange("p (n two) -> p n two", two=2)
                x3 = xb[:].rearrange("p (n two) -> p n two", two=2)[:, :, ::-1]
                nc.scalar.copy(z3, x3)  # pairswap on the idle ScalarE
                yb = yio.tile([P, S2], cdt, tag="yb", bufs=2)
                nc.vector.tensor_tensor(yb[:], xb[:], a_w, mult)
                nc.vector.tensor_tensor(zb[:], zb[:], b_w, mult)
                nc.vector.tensor_tensor(yb[:], yb[:], zb[:], add)
                dst = y_out[jj * 2 * P : (jj + 1) * 2 * P, :].rearrange(
                    "(b p) s -> p b s", b=2
                )
                nc.gpsimd.dma_start(dst, yb[:].rearrange("p (b s) -> p b s", b=2))
            total = 0  # dma2 handled; skip the generic per-tile loop

        else:
            total = N_TILES * repeats
        PRE = 3  # SWDGE load-prefetch depth: plain gpsimd.dma_start is
        # pinned to SWDGE queue 0 and the Pool NX executes in program
        # order, so loads must be emitted AHEAD of the store whose sem
        # wait would otherwise block their descriptor generation.
        _xbs = {}

        def _cast_load(j):
            jj = j % N_TILES
            xb = xio.tile([P, SIZE], cdt, tag="xb", bufs=PRE + 2, name="xb")
            nc.gpsimd.dma_start(xb[:], x_in[jj * P : (jj + 1) * P, :])
            _xbs[j] = xb

        if mode in ("dma", "dmal", "copy16"):
            for j in range(min(PRE, total)):
                _cast_load(j)

        for i in range(total):
            rows = slice((i % N_TILES) * P, (i % N_TILES + 1) * P)

            if mode == "copy":
                # DMA-roofline probe: f32 in/out, no compute.
                xt = xio.tile([P, SIZE], f32, tag="xt")
                nc.sync.dma_start(xt[:], x_in[rows, :])
                nc.gpsimd.dma_start(y_out[rows, :], xt[:])
                continue

            if mode == "copyss":
                # 1-ring probe: loads AND stores on the same HWDGE-SP ring.
                xt = xio.tile([P, SIZE], f32, tag="xt")
                nc.sync.dma_start(xt[:], x_in[rows, :])
                nc.sync.dma_start(y_out[rows, :], xt[:])
                continue

            if mode == "copy2h":
                # 2-HWDGE-ring probe: loads on SP ring, stores on ACT ring.
                xt = xio.tile([P, SIZE], f32, tag="xt")
                nc.sync.dma_start(xt[:], x_in[rows, :])
                nc.scalar.dma_start(y_out[rows, :], xt[:])
                continue

            if mode == "copy3":
                # 3-ring probe: loads alternate SP/ACT rings, stores SWDGE.
                xt = xio.tile([P, SIZE], f32, tag="xt")
                eng = nc.sync if i % 2 == 0 else nc.scalar
                eng.dma_start(xt[:], x_in[rows, :])
                nc.gpsimd.dma_start(y_out[rows, :], xt[:])
                continue

            if mode == "copy16":
                # cast-DMA roofline probe: f32->cdt load, cdt->f32 store.
                if i + PRE < total:
                    _cast_load(i + PRE)
                xb = _xbs.pop(i)
                nc.gpsimd.dma_start(y_out[rows, :], xb[:])
                continue

            if mode in ("dma", "dmal"):
                # 16-bit compute, casts ride the DMAs.
                if i + PRE < total:
                    _cast_load(i + PRE)
                xb = _xbs.pop(i)
                zb = zio.tile([P, SIZE], cdt, tag="zb")
                z3 = zb[:].rearrange("p (n two) -> p n two", two=2)
                x3 = xb[:].rearrange("p (n two) -> p n two", two=2)[:, :, ::-1]
                nc.scalar.copy(z3, x3)  # pairswap on the idle ScalarE
                if mode == "dma":
                    yb = yio.tile([P, SIZE], cdt, tag="yb")
                    nc.vector.tensor_tensor(yb[:], xb[:], a_w, mult)
                    nc.vector.tensor_tensor(zb[:], zb[:], b_w, mult)
                    nc.vector.tensor_tensor(yb[:], yb[:], zb[:], add)
                    nc.gpsimd.dma_start(y_out[rows, :], yb[:])
                else:
                    # dmal: final add writes f32 (1x DVE), store on HWDGE so
                    # the Pool stream carries only never-stalling loads.
                    yb = zio.tile([P, SIZE], cdt, tag="yb")
                    nc.vector.tensor_tensor(yb[:], xb[:], a_w, mult)
                    nc.vector.tensor_tensor(zb[:], zb[:], b_w, mult)
                    yt = yio.tile([P, SIZE], f32, tag="yt")
                    nc.vector.tensor_tensor(yt[:], yb[:], zb[:], add)
                    nc.sync.dma_start(y_out[rows, :], yt[:])
                continue

            if mode == "hyb":
                # Probe-measured best DMA topology (HWDGE loads + SWDGE
                # stores, like the copy probe) with 16-bit compute: f32
                # loads on the SP ring, BOTH casts on ScalarE (own SBUF
                # port - no DVE/GPSIMD port-pair lock), f16 DVE ops,
                # casting stores on SWDGE (Pool stream = stores only, so
                # loads prefetch freely like the f32 baseline).
                xt = xio.tile([P, SIZE], f32, tag="xt", bufs=3)
                nc.sync.dma_start(xt[:], x_in[rows, :])
                xb = xio.tile([P, SIZE], cdt, tag="xb", bufs=2)
                nc.scalar.copy(xb[:], xt[:])  # cast-in on ScalarE
                zb = zio.tile([P, SIZE], cdt, tag="zb", bufs=2)
                z3 = zb[:].rearrange("p (n two) -> p n two", two=2)
                x3 = xt[:].rearrange("p (n two) -> p n two", two=2)[:, :, ::-1]
                nc.scalar.copy(z3, x3)  # swap + cast on ScalarE
                yb = yio.tile([P, SIZE], cdt, tag="yb", bufs=3)
                nc.vector.tensor_tensor(yb[:], xb[:], a_w, mult)
                nc.vector.tensor_tensor(zb[:], zb[:], b_w, mult)
                nc.vector.tensor_tensor(yb[:], yb[:], zb[:], add)
                nc.gpsimd.dma_start(y_out[rows, :], yb[:])
                continue

            if mode == "eng":
                # fallback: plain f32 DMAs, casts on DVE/ScalarE.
                xt = xio.tile([P, SIZE], f32, tag="xt", bufs=3)
                nc.sync.dma_start(xt[:], x_in[rows, :])
                xb = xio.tile([P, SIZE], cdt, tag="xb", bufs=2)
                nc.vector.tensor_copy(xb[:], xt[:])  # f32 copy 2x_2P
                zb = zio.tile([P, SIZE], cdt, tag="zb", bufs=2)
                z3 = zb[:].rearrange("p (n two) -> p n two", two=2)
                x3 = xt[:].rearrange("p (n two) -> p n two", two=2)[:, :, ::-1]
                nc.scalar.copy(z3, x3)  # swap + cast on ScalarE
                yb = zio.tile([P, SIZE], cdt, tag="yb", bufs=2)
                nc.vector.tensor_tensor(yb[:], xb[:], a_w, mult)
                nc.vector.tensor_tensor(zb[:], zb[:], b_w, mult)
                nc.vector.tensor_tensor(yb[:], yb[:], zb[:], add)
                yt = yio.tile([P, SIZE], f32, tag="yt", bufs=3)
                nc.scalar.copy(yt[:], yb[:])  # cast back on ScalarE
                # HWDGE store: keeps GPSIMD off the SBUF port pair that the
                # DVE 2x_2P cast op lock-holds.
                nc.sync.dma_start(y_out[rows, :], yt[:])
                continue

            if mode == "f32":
                # legacy all-f32 path (baseline "swst" math, B' packing):
                # y = x*A; z = pairswap-copy(x) on ScalarE; z *= B'; y += z.
                xt = xio.tile([P, SIZE], f32, tag="xt", bufs=3)
                nc.sync.dma_start(xt[:], x_in[rows, :])
                zt = zio.tile([P, SIZE], f32, tag="zt", bufs=2)
                z3 = zt[:].rearrange("p (n two) -> p n two", two=2)
                x3 = xt[:].rearrange("p (n two) -> p n two", two=2)[:, :, ::-1]
                nc.scalar.copy(z3, x3)
                yt = yio.tile([P, SIZE], f32, tag="yt")
                nc.vector.tensor_tensor(yt[:], xt[:], a_w, mult)
                nc.vector.tensor_tensor(zt[:], zt[:], b_w, mult)
                nc.vector.tensor_tensor(yt[:], yt[:], zt[:], add)
                nc.gpsimd.dma_start(y_out[rows, :], yt[:])
                continue

            raise ValueError(mode)

    nc.compile()
    return nc


def _get_nc(repeats: int = 1, **kwargs):
    key = ("nc", repeats, tuple(sorted(kwargs.items())))
    if key not in _CACHE:
        _CACHE[key] = _build_program(repeats, **kwargs)
    return _CACHE[key]


def fold_weights(W: np.ndarray) -> np.ndarray:
    """Compose the 12 stacked per-pair 2x2 layers into one, in float64.

    Returns wf [1, 2*SIZE] float32: full-width A (diag: M00,M11 interleaved)
    followed by full-width B' (cross terms pre-swapped so that
    y = x*A + pairswap(x)*B'): B'[2n] = M[n,1,0], B'[2n+1] = M[n,0,1]."""
    Wd = W.astype(np.float64)  # [12, HALF, 2, 2]
    M = Wd[0]
    for l in range(1, Wd.shape[0]):
        M = np.einsum("nij,njk->nik", M, Wd[l])
    M = M.astype(np.float32)  # [HALF, 2, 2]
    a_full = np.stack([M[:, 0, 0], M[:, 1, 1]], axis=1).reshape(SIZE)
    b_full = np.stack([M[:, 1, 0], M[:, 0, 1]], axis=1).reshape(SIZE)
    wf = np.concatenate([a_full, b_full])
    return np.ascontiguousarray(wf.reshape(1, 2 * SIZE))


def _run(x: np.ndarray, W: np.ndarray, run_kwargs=None, **build_kwargs):
    """Shard, run on the 8 cores, gather. Returns (output, BassKernelResults)."""
    from concourse.bass_utils import run_bass_kernel_spmd

    assert x.shape == (BATCH, SIZE) and W.shape == (LOG_N, HALF, 2, 2)
    x = np.ascontiguousarray(x, dtype=np.float32)
    wf = fold_weights(np.asarray(W))

    nc = _get_nc(**build_kwargs)
    in_maps = [
        {"x": x[c * ROWS_PER_CORE : (c + 1) * ROWS_PER_CORE], "wf": wf}
        for c in range(N_CORES)
    ]
    res = run_bass_kernel_spmd(
        nc, in_maps, core_ids=list(range(N_CORES)), **(run_kwargs or {})
    )
    out = np.concatenate([res.results[c]["y"] for c in range(N_CORES)], axis=0)
    return out, res


def kernel(x: np.ndarray, W: np.ndarray) -> np.ndarray:
    return _run(x, W)[0]
